# revision 31
# baseline (speedup 1.0000x reference)
"""NodeAttention Trainium2 kernel (per-core program, SPMD over 8 cores).

v2.1 strategy (per core, i-block of NI=96 query rows):
- pair data pre-cast to bf16 on host (halves HBM traffic), loaded [c, i, j]
  via DRAM->SBUF xbar transpose DMA (one pass, no SWDGE, no SBUF->SBUF hop).
- bias projection as streaming matmuls with stationary wext [c, 32]
  (cols 0-7 head dots, col 8 mean, cols 16-23 -s_h/C so row 16+h of the
  result is -s_h*mu directly) + sqsel [c, 32] (col 10 = sum of squares over
  gpsimd-squared tiles), 4-way column-tiled: the four 32-row PSUM partition
  groups hold the four j-subblocks of a 128-j block.
- one DVE StreamTranspose per (ic, jb) turns the [sigma, (i, j)] PSUM block
  into DT [j, (i, sigma)] f32 directly.
- scalar ACT table thrash avoided: sqrt and exp batched per-ic (they live in
  different ACT tables; per-block alternation costs 1.3us per switch).
- t_h folded into sim via an extra contraction row (kT row 32 = t_h,
  qT row 32 = 1); softmax normalizer via ones column in V; logits bounded
  (~9) so no max-subtraction needed.
- ic-major loops so compute chases the 6 chunked transpose DMAs; att runs
  per 32-i group as E completes, with self-contained per-(g,jb) PSUM tiles.
"""
import numpy as np
from contextlib import ExitStack

import concourse.bass as bass
import concourse.tile as tile
from concourse import mybir
from concourse.masks import make_identity

f32 = mybir.dt.float32
fp16 = mybir.dt.float16
bf16 = mybir.dt.bfloat16

N = 768          # sequence length (j axis, also full i)
C = 128          # pair channels
H = 8            # heads
D = 32           # head dim
INNER = 256      # H*D
ND = 256         # node dim
NJB = N // 128   # 6 j-blocks
ICH = 16         # i-chunk per transpose DMA / dots tile
EPS = 1e-5


def _bc(ap2d: bass.AP, h: int) -> bass.AP:
    """[P, F] -> [P, h, F] with step-0 broadcast over the middle dim."""
    ap = list(ap2d.ap)
    assert len(ap) == 2
    return bass.AP(ap2d.tensor, ap2d.offset, [ap[0], [0, h], ap[1]])


def _bc_inner(ap2d: bass.AP, n: int) -> bass.AP:
    """[P, F] -> [P, F, n] with step-0 broadcast over the last dim."""
    ap = list(ap2d.ap)
    assert len(ap) == 2
    return bass.AP(ap2d.tensor, ap2d.offset, [ap[0], ap[1], [0, n]])


def build_nc(NI=96, n_cores=8, upto='full'):
    NIC = NI // ICH
    nc = bass.Bass("TRN2", target_bir_lowering=False, debug=False,
                   num_devices=n_cores)
    pairb = nc.dram_tensor("pairb", [NI, N, C], bf16, kind="ExternalInput").ap()
    node = nc.dram_tensor("node", [N, ND], f32, kind="ExternalInput").ap()
    nodeq = nc.dram_tensor("nodeq", [NI, ND], f32, kind="ExternalInput").ap()
    maskqb = nc.dram_tensor("maskqb", [NI, N], bf16, kind="ExternalInput").ap()
    wsq = nc.dram_tensor("wsq", [C, 64], bf16, kind="ExternalInput").ap()
    # wnode cols: [Wq*scale | Wk | Wv | Wg]
    wnode = nc.dram_tensor("wnode", [ND, 4 * INNER], bf16, kind="ExternalInput").ap()
    wout = nc.dram_tensor("wout", [INNER, ND], bf16, kind="ExternalInput").ap()
    cvec = nc.dram_tensor("cvec", [1, 3 * ND + INNER + 16], f32,
                          kind="ExternalInput").ap()
    tbc = nc.dram_tensor("tbc", [1, H, N], bf16, kind="ExternalInput").ap()
    y_out = nc.dram_tensor("y", [NI, ND], f32, kind="ExternalOutput").ap()
    dbg = nc.dram_tensor("dbg", [128, 1024], f32, kind="ExternalOutput").ap() \
        if upto != 'full' else None

    with tile.TileContext(nc) as tc, ExitStack() as ctx:
        const = ctx.enter_context(tc.tile_pool(name="const", bufs=1))
        persist = ctx.enter_context(tc.tile_pool(name="persist", bufs=1))

        # ---- small input DMAs first (sync FIFO runs ahead of TP transposes) ----
        wsq_sb = const.tile([C, 64], bf16)
        nc.gpsimd.dma_start(out=wsq_sb[:], in_=wsq)
        wext_sb = wsq_sb
        sqsel_sb = None
        wout_sb = const.tile([128, 2, ND], bf16)
        nc.gpsimd.dma_start(out=wout_sb[:],
                            in_=wout.rearrange("(kt p) c -> p kt c", p=128))

        # persistent node-derived tensors
        kT_sb = persist.tile([33, H, N], bf16)       # rows 0-31 k^T, row 32 t_h
        qT_sb = persist.tile([33, H, NI], bf16)      # rows 0-31 q^T, row 32 ones
        Vx_sb = persist.tile([128, NJB, H, D + 1], bf16)  # v + ones col
        m01T_sb = persist.tile([128, NJB, NI], bf16)      # mask^T [j', jb, i]
        sig_sb = persist.tile([NI, INNER], f32)           # sigmoid(g)
        sim_sb = persist.tile([128, NJB, H, NI], bf16)    # sim + t_h
        att_acc = persist.tile([NI, H, D + 1], f32)
        TP = persist.tile([128, NI, N], bf16)             # pair [c, i, j]

        nc.gpsimd.dma_start(out=kT_sb[32:33, :, :], in_=tbc)
        # mask transpose via xbar: [(i, jb), j'] -> [j', (i, jb)]
        m01T_dma = persist.tile([128, NI, NJB], bf16)  # [j', i, jb] from xbar
        nc.sync.dma_start_transpose(
            m01T_dma[:],
            maskqb.rearrange("i (jb jj) -> (i jb) jj", jj=128))
        md_ap = m01T_dma[:]
        nc.vector.tensor_copy(
            m01T_sb[:],
            bass.AP(md_ap.tensor, md_ap.offset,
                    [list(md_ap.ap)[0], [1, NJB], [NJB, NI]]))

        with tc.tile_pool(name="nodep", bufs=1) as npool, \
             tc.tile_pool(name="node_ps", bufs=1, space="PSUM") as nps:
            xq = npool.tile([NI, ND], f32)
            nc.gpsimd.dma_start(out=xq[:], in_=nodeq)
            x_all = npool.tile([128, N // 128, ND], f32)
            nc.gpsimd.dma_start(out=x_all[:],
                                in_=node.rearrange("(t p) c -> p t c", p=128))
            wn_sb = npool.tile([128, 2, 4 * INNER], bf16)
            nc.gpsimd.dma_start(out=wn_sb[:],
                                in_=wnode.rearrange("(kt p) c -> p kt c", p=128))

            # ---- the big pair transposes: bf16 DRAM -> SBUF [c, i, j] ----
            DCH = 16
            for dc in range(NI // DCH):
                i0 = dc * DCH
                nc.sync.dma_start_transpose(
                    TP[:, i0:i0 + DCH, :],
                    pairb[i0:i0 + DCH].rearrange("i j c -> (i j) c"))

            # ---- constants ----
            ident = const.tile([128, 128], f32)
            make_identity(nc, ident[:])
            eps_sb = const.tile([128, 1], f32)
            nc.vector.memset(eps_sb[:], EPS)

            cvec_sb = const.tile([128, 3 * ND + INNER + 16], f32)
            cv_b = bass.AP(cvec.tensor, cvec.offset,
                           [[0, 128]] + list(cvec.ap)[1:])
            nc.gpsimd.dma_start(out=cvec_sb[:], in_=cv_b)
            lnw_sb = cvec_sb[:, 0:ND]
            lnb_sb = cvec_sb[:, ND:2 * ND]
            bg_sb = cvec_sb[:, 2 * ND:2 * ND + INNER]
            bout_sb = cvec_sb[:, 2 * ND + INNER:3 * ND + INNER]

            # ---- node phase ----
            stats = npool.tile([128, 6], f32)
            mv = npool.tile([128, 2], f32)
            sd = npool.tile([128, 1], f32)
            rln = npool.tile([128, 1], f32)

            def layernorm_tile(xt, nrows):
                nc.vector.bn_stats(out=stats[:nrows, :], in_=xt)
                nc.vector.bn_aggr(out=mv[:nrows, :], in_=stats[:nrows, :])
                nc.scalar.activation(sd[:nrows, :], mv[:nrows, 1:2],
                                     mybir.ActivationFunctionType.Sqrt,
                                     bias=eps_sb[:nrows, :])
                nc.vector.reciprocal(rln[:nrows, :], sd[:nrows, :])
                nc.vector.tensor_scalar(out=xt, in0=xt,
                                        scalar1=mv[:nrows, 0:1],
                                        scalar2=rln[:nrows, :],
                                        op0=mybir.AluOpType.subtract,
                                        op1=mybir.AluOpType.mult)
                nc.vector.tensor_mul(xt, xt, lnw_sb[:nrows, :])
                nc.vector.tensor_add(xt, xt, lnb_sb[:nrows, :])

            for t in range(N // 128):
                layernorm_tile(x_all[:, t, :], 128)
            layernorm_tile(xq[:], NI)

            xT_sb = npool.tile([128, 2, N], bf16)
            xqT_sb = npool.tile([128, 2, NI], bf16)
            for t in range(N // 128):
                for kt in range(2):
                    tp = nps.tile([128, 128], f32, tag="xpose")
                    nc.tensor.transpose(tp[:], x_all[:, t, kt * 128:(kt + 1) * 128],
                                        ident[:])
                    nc.scalar.copy(xT_sb[:, kt, t * 128:(t + 1) * 128], tp[:])
            for kt in range(2):
                tp = nps.tile([128, NI], f32, tag="xpose")
                nc.tensor.transpose(tp[:], xq[:, kt * 128:(kt + 1) * 128],
                                    ident[:NI, :NI])
                nc.scalar.copy(xqT_sb[:, kt, :], tp[:])

            for h in range(H):
                for n0 in range(0, N, 384):
                    kp = nps.tile([32, 384], f32, tag="kmm")
                    for kt in range(2):
                        nc.tensor.matmul(
                            kp[:],
                            lhsT=wn_sb[:, kt, INNER + h * D:INNER + (h + 1) * D],
                            rhs=xT_sb[:, kt, n0:n0 + 384],
                            start=(kt == 0), stop=(kt == 1))
                    nc.scalar.copy(kT_sb[0:32, h, n0:n0 + 384], kp[:])

            for jb in range(NJB):
                vp = nps.tile([128, INNER], f32, tag="vmm")
                for kt in range(2):
                    nc.tensor.matmul(vp[:], lhsT=xT_sb[:, kt, jb * 128:(jb + 1) * 128],
                                     rhs=wn_sb[:, kt, 2 * INNER:3 * INNER],
                                     start=(kt == 0), stop=(kt == 1))
                nc.scalar.copy(Vx_sb[:, jb, :, 0:D],
                               vp[:].rearrange("p (h d) -> p h d", h=H))
            nc.vector.memset(Vx_sb[:, :, :, D:D + 1], 1.0)

            for h in range(H):
                qp = nps.tile([32, NI], f32, tag="qmm")
                for kt in range(2):
                    nc.tensor.matmul(qp[:], lhsT=wn_sb[:, kt, h * D:(h + 1) * D],
                                     rhs=xqT_sb[:, kt, :],
                                     start=(kt == 0), stop=(kt == 1))
                nc.scalar.copy(qT_sb[0:32, h, :], qp[:])
            nc.vector.memset(qT_sb[32:33, :, :], 1.0)

            gp = nps.tile([NI, INNER], f32, tag="gmm")
            for kt in range(2):
                nc.tensor.matmul(gp[:], lhsT=xqT_sb[:, kt, :],
                                 rhs=wn_sb[:, kt, 3 * INNER:4 * INNER],
                                 start=(kt == 0), stop=(kt == 1))
            gt = npool.tile([NI, INNER], f32)
            nc.vector.tensor_add(gt[:], gp[:], bg_sb[:NI, :])
            nc.scalar.activation(sig_sb[:NI, :], gt[:],
                                 mybir.ActivationFunctionType.Sigmoid)

        # sim + t_h per jb (k=33 contraction: row 32 = t_h x ones)
        with tc.tile_pool(name="sim_ps", bufs=2, space="PSUM") as simp:
            for jb in range(NJB):
                # h-stride padded to 512B: no MM output crosses a PSUM bank
                sp = simp.tile([128, H, 128], f32, tag="simm")
                for h in range(H):
                    nc.tensor.matmul(sp[:, h, 0:NI],
                                     lhsT=kT_sb[:, h, jb * 128:(jb + 1) * 128],
                                     rhs=qT_sb[:, h, :])
                nc.vector.scalar_tensor_tensor(
                    out=sim_sb[:, jb, :, :], in0=_bc(m01T_sb[:, jb, :], H),
                    scalar=60.0, in1=sp[:, :, 0:NI],
                    op0=mybir.AluOpType.mult, op1=mybir.AluOpType.add)

        if upto == 'sim':
            nc.gpsimd.dma_start(out=dbg[:, 0:H * NI],
                                in_=sim_sb[:, 0, :, :].rearrange("p a b -> p (a b)"))
            nc.scalar.dma_start(out=y_out, in_=sig_sb[:NI, :ND])
            return nc

        # ================= pair path (ic-major) =================
        nc.vector.memset(att_acc[:], 0.0)
        with tc.tile_pool(name="att_ps", bufs=2, space="PSUM") as attps, \
             tc.tile_pool(name="d_ps", bufs=2, space="PSUM") as dps, \
             tc.tile_pool(name="tsq", bufs=3) as sqp, \
             tc.tile_pool(name="dt", bufs=2) as dtp, \
             tc.tile_pool(name="lgic", bufs=2) as lgp, \
             tc.tile_pool(name="eg", bufs=2) as egp, \
             tc.tile_pool(name="st", bufs=2) as stp:
            E_g = None
            for ic in range(NIC):
                i0 = ic * ICH
                if ic % 2 == 0:
                    E_g = egp.tile([128, NJB, H, 32], bf16, tag="eg")
                eslc = (ic % 2) * ICH  # i-half within the 32-i group
                Lg_ic = lgp.tile([128, NJB, H, ICH], bf16, tag="lg")

                # pass 1: dots/ss matmuls, transpose, variance, tl = dots - s*mu
                for jb in range(NJB):
                    j0 = jb * 128
                    tsq = sqp.tile([C, ICH, 128], bf16, tag="tsq")
                    if jb % 3 == 0:
                        nc.scalar.square(tsq[:], TP[:, i0:i0 + ICH, j0:j0 + 128])
                    elif jb % 3 == 1:
                        nc.vector.tensor_mul(tsq[:], TP[:, i0:i0 + ICH, j0:j0 + 128],
                                             TP[:, i0:i0 + ICH, j0:j0 + 128])
                    else:
                        nc.gpsimd.tensor_mul(tsq[:], TP[:, i0:i0 + ICH, j0:j0 + 128],
                                             TP[:, i0:i0 + ICH, j0:j0 + 128])

                    Dps = dps.tile([128, ICH, 32], f32, tag="dps")
                    Dflat = Dps[:].rearrange("p i s -> p (i s)")
                    for b in range(4):
                        nc.tensor.matmul(
                            Dflat[32 * b:32 * b + 32, :],
                            lhsT=wsq_sb[:, 0:32],
                            rhs=TP[:, i0:i0 + ICH, j0 + 32 * b:j0 + 32 * b + 32],
                            tile_position=(0, 32 * b), start=True, stop=False)
                    for b in range(4):
                        nc.tensor.matmul(
                            Dflat[32 * b:32 * b + 32, :],
                            lhsT=wsq_sb[:, 32:64],
                            rhs=tsq[:, :, 32 * b:32 * b + 32],
                            tile_position=(0, 32 * b), start=False, stop=True)

                    DT = dtp.tile([128, ICH, 32], f32, tag="dt")
                    nc.vector.transpose(DT[:].rearrange("p i s -> p (i s)"),
                                        Dflat)

                    if upto == 'dt' and ic == 0 and jb == 0:
                        nc.gpsimd.dma_start(
                            out=dbg[:, 0:ICH * 32],
                            in_=DT[:].rearrange("p i s -> p (i s)"))
                        nc.scalar.dma_start(out=y_out, in_=sig_sb[:NI, :ND])

                    dt_ap = DT[:]
                    dots_hi = bass.AP(dt_ap.tensor, dt_ap.offset,
                                      [list(dt_ap.ap)[0], [1, H], [32, ICH]])
                    mu_ap = DT[:, :, 8]
                    ss_ap = DT[:, :, 10]
                    m2 = stp.tile([128, ICH], f32, tag="m2")
                    nc.vector.tensor_mul(m2[:], mu_ap, mu_ap)
                    var = stp.tile([128, ICH], f32, tag="var")
                    nc.vector.scalar_tensor_tensor(
                        out=var[:], in0=ss_ap, scalar=1.0 / C, in1=m2[:],
                        op0=mybir.AluOpType.mult, op1=mybir.AluOpType.subtract)
                    # sqrt per block: 'square'/'sqrt' share ACT tables with each
                    # other; only the per-ic sqrt->exp boundary reloads tables
                    sdp = stp.tile([128, ICH], f32, tag="sdp")
                    nc.scalar.activation(sdp[:], var[:],
                                         mybir.ActivationFunctionType.Sqrt,
                                         bias=eps_sb[:])
                    r = stp.tile([128, ICH], f32, tag="r")
                    nc.vector.reciprocal(r[:], sdp[:])
                    # logits = dots*r + sim (LN shift + mask + t_h folded in)
                    nc.vector.tensor_mul(Lg_ic[:, jb, :, :], dots_hi,
                                         _bc(r[:], H))
                    nc.vector.tensor_add(Lg_ic[:, jb, :, :], Lg_ic[:, jb, :, :],
                                         sim_sb[:, jb, :, i0:i0 + ICH])

                # batched exp (one ACT table visit per ic); mask already in sim
                nc.scalar.activation(E_g[:, :, :, eslc:eslc + ICH], Lg_ic[:],
                                     mybir.ActivationFunctionType.Exp)

                # att for the completed 32-i group
                if ic % 2 == 1:
                    g = ic // 2
                    for jb in range(NJB):
                        at = attps.tile([32, H, D + 1], f32, tag="attjb")
                        for h in range(H):
                            nc.tensor.matmul(at[:, h, :],
                                             lhsT=E_g[:, jb, h, :],
                                             rhs=Vx_sb[:, jb, h, :])
                        nc.vector.tensor_add(att_acc[32 * g:32 * g + 32, :, :],
                                             att_acc[32 * g:32 * g + 32, :, :],
                                             at[:])

        if upto == 'att':
            nc.scalar.dma_start(
                out=dbg[0:NI, 0:H * (D + 1)],
                in_=att_acc[:].rearrange("p a b -> p (a b)"))
            nc.scalar.dma_start(out=y_out, in_=sig_sb[:NI, :ND])
            return nc
        if upto == 'dt':
            return nc

        # ---- finalize (pair pools closed; only persistent tiles used) ----
        with tc.tile_pool(name="fin", bufs=1) as fin, \
             tc.tile_pool(name="fin_ps", bufs=2, space="PSUM") as finp:
            den_r = fin.tile([NI, H], f32)
            nc.vector.reciprocal(den_r[:], att_acc[:, :, D])
            gated = fin.tile([NI, INNER], f32)
            nc.vector.tensor_mul(gated[:].rearrange("p (h d) -> p h d", h=H),
                                 att_acc[:, :, 0:D], _bc_inner(den_r[:], D))
            nc.vector.tensor_mul(gated[:], gated[:], sig_sb[:NI, :])

            gT_sb = fin.tile([128, 2, NI], bf16)
            for kt in range(2):
                tp = finp.tile([128, NI], f32, tag="gpose")
                nc.tensor.transpose(tp[:], gated[:, kt * 128:(kt + 1) * 128],
                                    ident[:NI, :NI])
                nc.vector.tensor_copy(gT_sb[:, kt, :], tp[:])

            y_ps = finp.tile([NI, ND], f32, tag="ymm")
            for kt in range(2):
                nc.tensor.matmul(y_ps[:], lhsT=gT_sb[:, kt, :],
                                 rhs=wout_sb[:, kt, :],
                                 start=(kt == 0), stop=(kt == 1))
            y_sb = fin.tile([NI, ND], f32)
            nc.vector.tensor_add(y_sb[:], y_ps[:], bout_sb[:NI, :])
            nc.scalar.dma_start(out=y_out, in_=y_sb[:])

    return nc


def host_prep(inputs, NI=96, n_cores=8):
    """Slice/fold FULL inputs into per-core in_maps."""
    import ml_dtypes
    node_feats = np.asarray(inputs["node_feats"])[0]      # [N, ND]
    pair_feats = np.asarray(inputs["pair_feats"])[0]      # [N, N, C]
    mask = np.asarray(inputs["mask"])[0]                  # [N, N] bool
    lnw = np.asarray(inputs["ln_node_w"]).reshape(1, ND)
    lnb = np.asarray(inputs["ln_node_b"]).reshape(1, ND)
    lpw = np.asarray(inputs["ln_pair_w"])                 # [C]
    lpb = np.asarray(inputs["ln_pair_b"])                 # [C]
    w_qkv = np.asarray(inputs["w_qkv"])                   # [ND, 3*INNER]
    w_g = np.asarray(inputs["w_g"])                       # [ND, INNER]
    b_g = np.asarray(inputs["b_g"]).reshape(1, INNER)
    w_bias = np.asarray(inputs["w_bias"])                 # [C, H]
    w_out = np.asarray(inputs["w_out"])                   # [INNER, ND]
    b_out = np.asarray(inputs["b_out"]).reshape(1, ND)

    Wp = lpw[:, None] * w_bias                            # [C, H]
    s_h = Wp.sum(0)
    t_h = (lpb[:, None] * w_bias).sum(0)
    wsq = np.zeros((C, 64), np.float32)
    wsq[:, 0:H] = Wp - s_h[None, :] / C   # LN shift folded: p.(W - s/C) = dot - s*mu
    wsq[:, 8] = 1.0 / C
    wsq[:, 32 + 10] = 1.0                 # sum-of-squares selector
    wsq = wsq.astype(ml_dtypes.bfloat16)

    scale = D ** -0.5
    wnode = np.concatenate([w_qkv[:, 0:INNER] * scale,
                            w_qkv[:, INNER:2 * INNER],
                            w_qkv[:, 2 * INNER:3 * INNER],
                            w_g], axis=1).astype(ml_dtypes.bfloat16)
    woutb = w_out.astype(ml_dtypes.bfloat16)
    sth = np.stack([-s_h, t_h]).astype(np.float32)        # [2, H]
    tbc = np.broadcast_to((t_h - 60.0).astype(np.float32).astype(
        ml_dtypes.bfloat16)[None, :, None], (1, H, N)).copy()

    pair_bf = pair_feats.astype(ml_dtypes.bfloat16)
    mask_bf = mask.astype(ml_dtypes.bfloat16)

    cvec = np.concatenate([lnw.ravel(), lnb.ravel(), b_g.ravel(),
                           b_out.ravel(), sth.ravel()]).astype(np.float32)[None, :]
    shared = dict(node=node_feats.astype(np.float32), wsq=wsq,
                  wnode=wnode, wout=woutb, cvec=cvec, tbc=tbc)
    in_maps = []
    for c in range(n_cores):
        i0 = c * NI
        in_maps.append(dict(
            pairb=np.ascontiguousarray(pair_bf[i0:i0 + NI]),
            nodeq=np.ascontiguousarray(node_feats[i0:i0 + NI]).astype(np.float32),
            maskqb=np.ascontiguousarray(mask_bf[i0:i0 + NI]),
            **shared))
    return in_maps


def split_sync_waits(nc, limit=1):
    """Walrus (this container's neuronxcc) rejects instructions carrying more
    than `limit` sem waits. Hoist excess waits onto per-engine carrier drains
    inserted just before the offending instruction."""
    n_split = 0
    for f in nc.m.functions:
        for bb in f.blocks:
            out = []
            for inst in bb.instructions:
                si = inst.sync_info
                waits = list(si.on_wait) if si and si.on_wait else []
                if len(waits) > limit:
                    extra, keep = waits[:-limit], waits[-limit:]
                    for ci in range(0, len(extra), limit):
                        chunk = extra[ci:ci + limit]
                        nd = mybir.InstDrain(name=f"{inst.name}-wsplit{ci}",
                                             ins=[], outs=[])
                        nd.engine = inst.engine
                        nd.sync_info = mybir.SyncInfo(on_wait=chunk, on_update=[])
                        out.append(nd)
                        n_split += 1
                    si.on_wait = keep
                out.append(inst)
            bb.instructions = out
    return n_split


_CACHED = {}


def kernel(**inputs):
    """Full-input entry point: shards over 8 NeuronCores, returns full output."""
    NC_CORES = 8
    NI = N // NC_CORES
    from concourse.bass_utils import run_bass_kernel_spmd

    in_maps = host_prep(inputs, NI=NI, n_cores=NC_CORES)
    if "nc" not in _CACHED:
        nc = build_nc(NI=NI, n_cores=NC_CORES)
        split_sync_waits(nc)
        _CACHED["nc"] = nc
    res = run_bass_kernel_spmd(_CACHED["nc"], in_maps, list(range(NC_CORES)))
    y = np.concatenate([res.results[c]["y"] for c in range(NC_CORES)], axis=0)
    return y[None].astype(np.float32)


# revision 32
# speedup vs baseline: 1.3477x; 1.3477x over previous
"""NodeAttention Trainium2 kernel (per-core program, SPMD over 8 cores).

v2.1 strategy (per core, i-block of NI=96 query rows):
- pair data pre-cast to bf16 on host (halves HBM traffic), loaded [c, i, j]
  via DRAM->SBUF xbar transpose DMA (one pass, no SWDGE, no SBUF->SBUF hop).
- bias projection as streaming matmuls with stationary wext [c, 32]
  (cols 0-7 head dots, col 8 mean, cols 16-23 -s_h/C so row 16+h of the
  result is -s_h*mu directly) + sqsel [c, 32] (col 10 = sum of squares over
  gpsimd-squared tiles), 4-way column-tiled: the four 32-row PSUM partition
  groups hold the four j-subblocks of a 128-j block.
- one DVE StreamTranspose per (ic, jb) turns the [sigma, (i, j)] PSUM block
  into DT [j, (i, sigma)] f32 directly.
- scalar ACT table thrash avoided: sqrt and exp batched per-ic (they live in
  different ACT tables; per-block alternation costs 1.3us per switch).
- t_h folded into sim via an extra contraction row (kT row 32 = t_h,
  qT row 32 = 1); softmax normalizer via ones column in V; logits bounded
  (~9) so no max-subtraction needed.
- ic-major loops so compute chases the 6 chunked transpose DMAs; att runs
  per 32-i group as E completes, with self-contained per-(g,jb) PSUM tiles.
"""
import numpy as np
from contextlib import ExitStack

import concourse.bass as bass
import concourse.tile as tile
from concourse import mybir
from concourse.masks import make_identity

f32 = mybir.dt.float32
fp16 = mybir.dt.float16
bf16 = mybir.dt.bfloat16

N = 768          # sequence length (j axis, also full i)
C = 128          # pair channels
H = 8            # heads
D = 32           # head dim
INNER = 256      # H*D
ND = 256         # node dim
NJB = N // 128   # 6 j-blocks
ICH = 16         # i-chunk per transpose DMA / dots tile
EPS = 1e-5


def _bc(ap2d: bass.AP, h: int) -> bass.AP:
    """[P, F] -> [P, h, F] with step-0 broadcast over the middle dim."""
    ap = list(ap2d.ap)
    assert len(ap) == 2
    return bass.AP(ap2d.tensor, ap2d.offset, [ap[0], [0, h], ap[1]])


def _bc_inner(ap2d: bass.AP, n: int) -> bass.AP:
    """[P, F] -> [P, F, n] with step-0 broadcast over the last dim."""
    ap = list(ap2d.ap)
    assert len(ap) == 2
    return bass.AP(ap2d.tensor, ap2d.offset, [ap[0], ap[1], [0, n]])


def build_nc(NI=96, n_cores=8, upto='full'):
    NIC = NI // ICH
    nc = bass.Bass("TRN2", target_bir_lowering=False, debug=False,
                   num_devices=n_cores)
    pairb = nc.dram_tensor("pairb", [NI, N, C], bf16, kind="ExternalInput").ap()
    node = nc.dram_tensor("node", [N, ND], f32, kind="ExternalInput").ap()
    nodeq = nc.dram_tensor("nodeq", [NI, ND], f32, kind="ExternalInput").ap()
    maskqb = nc.dram_tensor("maskqb", [NI, N], bf16, kind="ExternalInput").ap()
    wsq = nc.dram_tensor("wsq", [C, 64], bf16, kind="ExternalInput").ap()
    # wnode cols: [Wq*scale | Wk | Wv | Wg]
    wnode = nc.dram_tensor("wnode", [ND, 4 * INNER], bf16, kind="ExternalInput").ap()
    wout = nc.dram_tensor("wout", [INNER, ND], bf16, kind="ExternalInput").ap()
    cvec = nc.dram_tensor("cvec", [1, 3 * ND + INNER + 16], f32,
                          kind="ExternalInput").ap()
    tbc = nc.dram_tensor("tbc", [1, H, N], bf16, kind="ExternalInput").ap()
    y_out = nc.dram_tensor("y", [NI, ND], f32, kind="ExternalOutput").ap()
    dbg = nc.dram_tensor("dbg", [128, 1024], f32, kind="ExternalOutput").ap() \
        if upto != 'full' else None

    with tile.TileContext(nc) as tc, ExitStack() as ctx:
        const = ctx.enter_context(tc.tile_pool(name="const", bufs=1))
        persist = ctx.enter_context(tc.tile_pool(name="persist", bufs=1))

        # ---- small input DMAs first (sync FIFO runs ahead of TP transposes) ----
        wsq_sb = const.tile([C, 64], bf16)
        nc.gpsimd.dma_start(out=wsq_sb[:], in_=wsq)
        wext_sb = wsq_sb
        sqsel_sb = None
        wout_sb = const.tile([128, 2, ND], bf16)
        nc.gpsimd.dma_start(out=wout_sb[:],
                            in_=wout.rearrange("(kt p) c -> p kt c", p=128))

        # persistent node-derived tensors
        kT_sb = persist.tile([33, H, N], bf16)       # rows 0-31 k^T, row 32 t_h
        qT_sb = persist.tile([33, H, NI], bf16)      # rows 0-31 q^T, row 32 ones
        Vx_sb = persist.tile([128, NJB, H, D + 1], bf16)  # v + ones col
        m01T_sb = persist.tile([128, NJB, NI], bf16)      # mask^T [j', jb, i]
        sig_sb = persist.tile([NI, INNER], f32)           # sigmoid(g)
        sim_sb = persist.tile([128, NJB, H, NI], bf16)    # sim + t_h
        att_acc = persist.tile([NI, H, D + 1], f32)
        TP = persist.tile([128, NI, N], bf16)             # pair [c, i, j]

        nc.gpsimd.dma_start(out=kT_sb[32:33, :, :], in_=tbc)
        # mask transpose via xbar: [(i, jb), j'] -> [j', (i, jb)]
        m01T_dma = persist.tile([128, NI, NJB], bf16)  # [j', i, jb] from xbar
        nc.sync.dma_start_transpose(
            m01T_dma[:],
            maskqb.rearrange("i (jb jj) -> (i jb) jj", jj=128))
        md_ap = m01T_dma[:]
        nc.vector.tensor_copy(
            m01T_sb[:],
            bass.AP(md_ap.tensor, md_ap.offset,
                    [list(md_ap.ap)[0], [1, NJB], [NJB, NI]]))

        with tc.tile_pool(name="nodep", bufs=1) as npool, \
             tc.tile_pool(name="node_ps", bufs=1, space="PSUM") as nps:
            xq = npool.tile([NI, ND], f32)
            nc.gpsimd.dma_start(out=xq[:], in_=nodeq)
            x_all = npool.tile([128, N // 128, ND], f32)
            nc.gpsimd.dma_start(out=x_all[:],
                                in_=node.rearrange("(t p) c -> p t c", p=128))
            wn_sb = npool.tile([128, 2, 4 * INNER], bf16)
            nc.gpsimd.dma_start(out=wn_sb[:],
                                in_=wnode.rearrange("(kt p) c -> p kt c", p=128))

            # ---- the big pair transposes: bf16 DRAM -> SBUF [c, i, j] ----
            DCH = 16
            for dc in range(NI // DCH):
                i0 = dc * DCH
                nc.sync.dma_start_transpose(
                    TP[:, i0:i0 + DCH, :],
                    pairb[i0:i0 + DCH].rearrange("i j c -> (i j) c"))

            # ---- constants ----
            ident = const.tile([128, 128], f32)
            make_identity(nc, ident[:])
            eps_sb = const.tile([128, 1], f32)
            nc.vector.memset(eps_sb[:], EPS)

            cvec_sb = const.tile([128, 3 * ND + INNER + 16], f32)
            cv_b = bass.AP(cvec.tensor, cvec.offset,
                           [[0, 128]] + list(cvec.ap)[1:])
            nc.gpsimd.dma_start(out=cvec_sb[:], in_=cv_b)
            lnw_sb = cvec_sb[:, 0:ND]
            lnb_sb = cvec_sb[:, ND:2 * ND]
            bg_sb = cvec_sb[:, 2 * ND:2 * ND + INNER]
            bout_sb = cvec_sb[:, 2 * ND + INNER:3 * ND + INNER]

            # ---- node phase ----
            stats = npool.tile([128, 6], f32)
            mv = npool.tile([128, 2], f32)
            sd = npool.tile([128, 1], f32)
            rln = npool.tile([128, 1], f32)

            def layernorm_tile(xt, nrows):
                nc.vector.bn_stats(out=stats[:nrows, :], in_=xt)
                nc.vector.bn_aggr(out=mv[:nrows, :], in_=stats[:nrows, :])
                nc.scalar.activation(sd[:nrows, :], mv[:nrows, 1:2],
                                     mybir.ActivationFunctionType.Sqrt,
                                     bias=eps_sb[:nrows, :])
                nc.vector.reciprocal(rln[:nrows, :], sd[:nrows, :])
                nc.vector.tensor_scalar(out=xt, in0=xt,
                                        scalar1=mv[:nrows, 0:1],
                                        scalar2=rln[:nrows, :],
                                        op0=mybir.AluOpType.subtract,
                                        op1=mybir.AluOpType.mult)
                nc.vector.tensor_mul(xt, xt, lnw_sb[:nrows, :])
                nc.vector.tensor_add(xt, xt, lnb_sb[:nrows, :])

            for t in range(N // 128):
                layernorm_tile(x_all[:, t, :], 128)
            layernorm_tile(xq[:], NI)

            xT_sb = npool.tile([128, 2, N], bf16)
            xqT_sb = npool.tile([128, 2, NI], bf16)
            for t in range(N // 128):
                for kt in range(2):
                    tp = nps.tile([128, 128], f32, tag="xpose")
                    nc.tensor.transpose(tp[:], x_all[:, t, kt * 128:(kt + 1) * 128],
                                        ident[:])
                    nc.scalar.copy(xT_sb[:, kt, t * 128:(t + 1) * 128], tp[:])
            for kt in range(2):
                tp = nps.tile([128, NI], f32, tag="xpose")
                nc.tensor.transpose(tp[:], xq[:, kt * 128:(kt + 1) * 128],
                                    ident[:NI, :NI])
                nc.scalar.copy(xqT_sb[:, kt, :], tp[:])

            for h in range(H):
                for n0 in range(0, N, 384):
                    kp = nps.tile([32, 384], f32, tag="kmm")
                    for kt in range(2):
                        nc.tensor.matmul(
                            kp[:],
                            lhsT=wn_sb[:, kt, INNER + h * D:INNER + (h + 1) * D],
                            rhs=xT_sb[:, kt, n0:n0 + 384],
                            start=(kt == 0), stop=(kt == 1))
                    nc.scalar.copy(kT_sb[0:32, h, n0:n0 + 384], kp[:])

            for jb in range(NJB):
                vp = nps.tile([128, INNER], f32, tag="vmm")
                for kt in range(2):
                    nc.tensor.matmul(vp[:], lhsT=xT_sb[:, kt, jb * 128:(jb + 1) * 128],
                                     rhs=wn_sb[:, kt, 2 * INNER:3 * INNER],
                                     start=(kt == 0), stop=(kt == 1))
                nc.scalar.copy(Vx_sb[:, jb, :, 0:D],
                               vp[:].rearrange("p (h d) -> p h d", h=H))
            nc.vector.memset(Vx_sb[:, :, :, D:D + 1], 1.0)

            for h in range(H):
                qp = nps.tile([32, NI], f32, tag="qmm")
                for kt in range(2):
                    nc.tensor.matmul(qp[:], lhsT=wn_sb[:, kt, h * D:(h + 1) * D],
                                     rhs=xqT_sb[:, kt, :],
                                     start=(kt == 0), stop=(kt == 1))
                nc.scalar.copy(qT_sb[0:32, h, :], qp[:])
            nc.vector.memset(qT_sb[32:33, :, :], 1.0)

            gp = nps.tile([NI, INNER], f32, tag="gmm")
            for kt in range(2):
                nc.tensor.matmul(gp[:], lhsT=xqT_sb[:, kt, :],
                                 rhs=wn_sb[:, kt, 3 * INNER:4 * INNER],
                                 start=(kt == 0), stop=(kt == 1))
            gt = npool.tile([NI, INNER], f32)
            nc.vector.tensor_add(gt[:], gp[:], bg_sb[:NI, :])
            nc.scalar.activation(sig_sb[:NI, :], gt[:],
                                 mybir.ActivationFunctionType.Sigmoid)

        # sim + t_h per jb (k=33 contraction: row 32 = t_h x ones)
        with tc.tile_pool(name="sim_ps", bufs=2, space="PSUM") as simp:
            for jb in range(NJB):
                # h-stride padded to 512B: no MM output crosses a PSUM bank
                sp = simp.tile([128, H, 128], f32, tag="simm")
                for h in range(H):
                    nc.tensor.matmul(sp[:, h, 0:NI],
                                     lhsT=kT_sb[:, h, jb * 128:(jb + 1) * 128],
                                     rhs=qT_sb[:, h, :])
                nc.vector.scalar_tensor_tensor(
                    out=sim_sb[:, jb, :, :], in0=_bc(m01T_sb[:, jb, :], H),
                    scalar=60.0, in1=sp[:, :, 0:NI],
                    op0=mybir.AluOpType.mult, op1=mybir.AluOpType.add)

        if upto == 'sim':
            nc.gpsimd.dma_start(out=dbg[:, 0:H * NI],
                                in_=sim_sb[:, 0, :, :].rearrange("p a b -> p (a b)"))
            nc.scalar.dma_start(out=y_out, in_=sig_sb[:NI, :ND])
            return nc

        # ================= pair path (ic-major) =================
        nc.vector.memset(att_acc[:], 0.0)
        with tc.tile_pool(name="att_ps", bufs=2, space="PSUM") as attps, \
             tc.tile_pool(name="d_ps", bufs=2, space="PSUM") as dps, \
             tc.tile_pool(name="tsq", bufs=3) as sqp, \
             tc.tile_pool(name="dt", bufs=2) as dtp, \
             tc.tile_pool(name="lgic", bufs=2) as lgp, \
             tc.tile_pool(name="eg", bufs=2) as egp, \
             tc.tile_pool(name="st", bufs=2) as stp:
            E_g = None
            for ic in range(NIC):
                i0 = ic * ICH
                if ic % 2 == 0:
                    E_g = egp.tile([128, NJB, H, 32], bf16, tag="eg")
                eslc = (ic % 2) * ICH  # i-half within the 32-i group
                Lg_ic = lgp.tile([128, NJB, H, ICH], bf16, tag="lg")

                # pass 1: dots/ss matmuls, transpose, variance, tl = dots - s*mu
                for jb in range(NJB):
                    j0 = jb * 128
                    tsq = sqp.tile([C, ICH, 128], bf16, tag="tsq")
                    if jb % 2 == 0:
                        nc.scalar.square(tsq[:], TP[:, i0:i0 + ICH, j0:j0 + 128])
                    else:
                        nc.vector.tensor_mul(tsq[:], TP[:, i0:i0 + ICH, j0:j0 + 128],
                                             TP[:, i0:i0 + ICH, j0:j0 + 128])

                    Dps = dps.tile([128, ICH, 32], f32, tag="dps")
                    Dflat = Dps[:].rearrange("p i s -> p (i s)")
                    for b in range(4):
                        nc.tensor.matmul(
                            Dflat[32 * b:32 * b + 32, :],
                            lhsT=wsq_sb[:, 0:32],
                            rhs=TP[:, i0:i0 + ICH, j0 + 32 * b:j0 + 32 * b + 32],
                            tile_position=(0, 32 * b), start=True, stop=False)
                    for b in range(4):
                        nc.tensor.matmul(
                            Dflat[32 * b:32 * b + 32, :],
                            lhsT=wsq_sb[:, 32:64],
                            rhs=tsq[:, :, 32 * b:32 * b + 32],
                            tile_position=(0, 32 * b), start=False, stop=True)

                    DT = dtp.tile([128, ICH, 32], f32, tag="dt")
                    nc.vector.transpose(DT[:].rearrange("p i s -> p (i s)"),
                                        Dflat)

                    if upto == 'dt' and ic == 0 and jb == 0:
                        nc.gpsimd.dma_start(
                            out=dbg[:, 0:ICH * 32],
                            in_=DT[:].rearrange("p i s -> p (i s)"))
                        nc.scalar.dma_start(out=y_out, in_=sig_sb[:NI, :ND])

                    dt_ap = DT[:]
                    dots_hi = bass.AP(dt_ap.tensor, dt_ap.offset,
                                      [list(dt_ap.ap)[0], [1, H], [32, ICH]])
                    mu_ap = DT[:, :, 8]
                    ss_ap = DT[:, :, 10]
                    m2 = stp.tile([128, ICH], f32, tag="m2")
                    nc.vector.tensor_mul(m2[:], mu_ap, mu_ap)
                    var = stp.tile([128, ICH], f32, tag="var")
                    nc.vector.scalar_tensor_tensor(
                        out=var[:], in0=ss_ap, scalar=1.0 / C, in1=m2[:],
                        op0=mybir.AluOpType.mult, op1=mybir.AluOpType.subtract)
                    # sqrt per block: 'square'/'sqrt' share ACT tables with each
                    # other; only the per-ic sqrt->exp boundary reloads tables
                    sdp = stp.tile([128, ICH], f32, tag="sdp")
                    nc.scalar.activation(sdp[:], var[:],
                                         mybir.ActivationFunctionType.Sqrt,
                                         bias=eps_sb[:])
                    r = stp.tile([128, ICH], f32, tag="r")
                    nc.vector.reciprocal(r[:], sdp[:])
                    # logits = dots*r + sim (LN shift + mask + t_h folded in)
                    nc.vector.tensor_mul(Lg_ic[:, jb, :, :], dots_hi,
                                         _bc(r[:], H))
                    nc.vector.tensor_add(Lg_ic[:, jb, :, :], Lg_ic[:, jb, :, :],
                                         sim_sb[:, jb, :, i0:i0 + ICH])

                # batched exp (one ACT table visit per ic); mask already in sim
                nc.scalar.activation(E_g[:, :, :, eslc:eslc + ICH], Lg_ic[:],
                                     mybir.ActivationFunctionType.Exp)

                # att for the completed 32-i group
                if ic % 2 == 1:
                    g = ic // 2
                    for jb in range(NJB):
                        at = attps.tile([32, H, D + 1], f32, tag="attjb")
                        for h in range(H):
                            nc.tensor.matmul(at[:, h, :],
                                             lhsT=E_g[:, jb, h, :],
                                             rhs=Vx_sb[:, jb, h, :])
                        nc.vector.tensor_add(att_acc[32 * g:32 * g + 32, :, :],
                                             att_acc[32 * g:32 * g + 32, :, :],
                                             at[:])

        if upto == 'att':
            nc.scalar.dma_start(
                out=dbg[0:NI, 0:H * (D + 1)],
                in_=att_acc[:].rearrange("p a b -> p (a b)"))
            nc.scalar.dma_start(out=y_out, in_=sig_sb[:NI, :ND])
            return nc
        if upto == 'dt':
            return nc

        # ---- finalize (pair pools closed; only persistent tiles used) ----
        with tc.tile_pool(name="fin", bufs=1) as fin, \
             tc.tile_pool(name="fin_ps", bufs=2, space="PSUM") as finp:
            den_r = fin.tile([NI, H], f32)
            nc.vector.reciprocal(den_r[:], att_acc[:, :, D])
            gated = fin.tile([NI, INNER], f32)
            nc.vector.tensor_mul(gated[:].rearrange("p (h d) -> p h d", h=H),
                                 att_acc[:, :, 0:D], _bc_inner(den_r[:], D))
            nc.vector.tensor_mul(gated[:], gated[:], sig_sb[:NI, :])

            gT_sb = fin.tile([128, 2, NI], bf16)
            for kt in range(2):
                tp = finp.tile([128, NI], f32, tag="gpose")
                nc.tensor.transpose(tp[:], gated[:, kt * 128:(kt + 1) * 128],
                                    ident[:NI, :NI])
                nc.vector.tensor_copy(gT_sb[:, kt, :], tp[:])

            y_ps = finp.tile([NI, ND], f32, tag="ymm")
            for kt in range(2):
                nc.tensor.matmul(y_ps[:], lhsT=gT_sb[:, kt, :],
                                 rhs=wout_sb[:, kt, :],
                                 start=(kt == 0), stop=(kt == 1))
            y_sb = fin.tile([NI, ND], f32)
            nc.vector.tensor_add(y_sb[:], y_ps[:], bout_sb[:NI, :])
            nc.scalar.dma_start(out=y_out, in_=y_sb[:])

    return nc


def host_prep(inputs, NI=96, n_cores=8):
    """Slice/fold FULL inputs into per-core in_maps."""
    import ml_dtypes
    node_feats = np.asarray(inputs["node_feats"])[0]      # [N, ND]
    pair_feats = np.asarray(inputs["pair_feats"])[0]      # [N, N, C]
    mask = np.asarray(inputs["mask"])[0]                  # [N, N] bool
    lnw = np.asarray(inputs["ln_node_w"]).reshape(1, ND)
    lnb = np.asarray(inputs["ln_node_b"]).reshape(1, ND)
    lpw = np.asarray(inputs["ln_pair_w"])                 # [C]
    lpb = np.asarray(inputs["ln_pair_b"])                 # [C]
    w_qkv = np.asarray(inputs["w_qkv"])                   # [ND, 3*INNER]
    w_g = np.asarray(inputs["w_g"])                       # [ND, INNER]
    b_g = np.asarray(inputs["b_g"]).reshape(1, INNER)
    w_bias = np.asarray(inputs["w_bias"])                 # [C, H]
    w_out = np.asarray(inputs["w_out"])                   # [INNER, ND]
    b_out = np.asarray(inputs["b_out"]).reshape(1, ND)

    Wp = lpw[:, None] * w_bias                            # [C, H]
    s_h = Wp.sum(0)
    t_h = (lpb[:, None] * w_bias).sum(0)
    wsq = np.zeros((C, 64), np.float32)
    wsq[:, 0:H] = Wp - s_h[None, :] / C   # LN shift folded: p.(W - s/C) = dot - s*mu
    wsq[:, 8] = 1.0 / C
    wsq[:, 32 + 10] = 1.0                 # sum-of-squares selector
    wsq = wsq.astype(ml_dtypes.bfloat16)

    scale = D ** -0.5
    wnode = np.concatenate([w_qkv[:, 0:INNER] * scale,
                            w_qkv[:, INNER:2 * INNER],
                            w_qkv[:, 2 * INNER:3 * INNER],
                            w_g], axis=1).astype(ml_dtypes.bfloat16)
    woutb = w_out.astype(ml_dtypes.bfloat16)
    sth = np.stack([-s_h, t_h]).astype(np.float32)        # [2, H]
    tbc = np.broadcast_to((t_h - 60.0).astype(np.float32).astype(
        ml_dtypes.bfloat16)[None, :, None], (1, H, N)).copy()

    pair_bf = pair_feats.astype(ml_dtypes.bfloat16)
    mask_bf = mask.astype(ml_dtypes.bfloat16)

    cvec = np.concatenate([lnw.ravel(), lnb.ravel(), b_g.ravel(),
                           b_out.ravel(), sth.ravel()]).astype(np.float32)[None, :]
    shared = dict(node=node_feats.astype(np.float32), wsq=wsq,
                  wnode=wnode, wout=woutb, cvec=cvec, tbc=tbc)
    in_maps = []
    for c in range(n_cores):
        i0 = c * NI
        in_maps.append(dict(
            pairb=np.ascontiguousarray(pair_bf[i0:i0 + NI]),
            nodeq=np.ascontiguousarray(node_feats[i0:i0 + NI]).astype(np.float32),
            maskqb=np.ascontiguousarray(mask_bf[i0:i0 + NI]),
            **shared))
    return in_maps


def split_sync_waits(nc, limit=1):
    """Walrus (this container's neuronxcc) rejects instructions carrying more
    than `limit` sem waits. Hoist excess waits onto per-engine carrier drains
    inserted just before the offending instruction."""
    n_split = 0
    for f in nc.m.functions:
        for bb in f.blocks:
            out = []
            for inst in bb.instructions:
                si = inst.sync_info
                waits = list(si.on_wait) if si and si.on_wait else []
                if len(waits) > limit:
                    extra, keep = waits[:-limit], waits[-limit:]
                    for ci in range(0, len(extra), limit):
                        chunk = extra[ci:ci + limit]
                        nd = mybir.InstDrain(name=f"{inst.name}-wsplit{ci}",
                                             ins=[], outs=[])
                        nd.engine = inst.engine
                        nd.sync_info = mybir.SyncInfo(on_wait=chunk, on_update=[])
                        out.append(nd)
                        n_split += 1
                    si.on_wait = keep
                out.append(inst)
            bb.instructions = out
    return n_split


_CACHED = {}


def kernel(**inputs):
    """Full-input entry point: shards over 8 NeuronCores, returns full output."""
    NC_CORES = 8
    NI = N // NC_CORES
    from concourse.bass_utils import run_bass_kernel_spmd

    in_maps = host_prep(inputs, NI=NI, n_cores=NC_CORES)
    if "nc" not in _CACHED:
        nc = build_nc(NI=NI, n_cores=NC_CORES)
        split_sync_waits(nc)
        _CACHED["nc"] = nc
    res = run_bass_kernel_spmd(_CACHED["nc"], in_maps, list(range(NC_CORES)))
    y = np.concatenate([res.results[c]["y"] for c in range(NC_CORES)], axis=0)
    return y[None].astype(np.float32)
